# revision 5
# baseline (speedup 1.0000x reference)
"""Trainium2 Bass kernel for nn_DifferentiableLattice (gnn_message_passing).

Reference computation (per step, 9 steps):
    m = max(state)                         # global over (B, N)
    state = state @ P.T
    state = state * angle_factor * decay
    state = sigmoid(2*state - 1) * max(m, 0.1)
then out = sum_t softmax(step_weights)[t] * state_t   (incl. state_0 = x)

Kernel strategy (8 NeuronCores, data-parallel over batch):
  * Host precomputes W2 = 2*decay*diag(angle_factor) @ P  (512x512) and the
    softmax weights w[t]; shards x row-wise into 8 x [2048, 512].
  * On-chip state is the *unscaled* sigmoid output s~_t, kept transposed
    [cells(part), batch(free)] so each step's matmul output layout feeds the
    next step's matmul directly (contraction dim lands on partitions).
        raw_{t+1} = (c_{t-1} * W2) @ s~_t        (TensorE, fp32)
        s~_{t+1}  = sigmoid(raw - 1)             (ScalarE, scale/bias folded)
        acc      += (w_t * c_{t-1}) * s~_t       (GPSIMD fused scalar_tensor_tensor)
        pmax      = per-partition max of s~_t    (VectorE reduce)
    where c_t = max(c_{t-1} * gmax(s~_t), 0.1) and gmax is a global max across
    all 8 shards: per-core partition_all_reduce + a tiny AllReduce(max)
    collective per step, overlapped with the next step's matmuls.
  * The per-step weight rescale c*W2 touches only 1MB (ScalarE), not the 4MB
    state.  PE transposes (128x128 via identity matmul) convert x -> x^T at
    the start and acc -> out at the end.
"""

import os
import sys

import numpy as np

sys.path.insert(0, "/opt/trn_rl_repo")

from contextlib import ExitStack

import concourse.bacc as bacc
import concourse.bass as bass
import concourse.bass_isa as bass_isa
import concourse.mybir as mybir
import concourse.tile as tile
from concourse.bass_utils import run_bass_kernel_spmd

F32 = mybir.dt.float32
ALU = mybir.AluOpType
AX = mybir.AxisListType
ACTF = mybir.ActivationFunctionType

N_CELLS = 512
BATCH = 16384
N_CORES = 8
BSH = BATCH // N_CORES          # 2048 batch rows per core
KT = N_CELLS // 128             # 4 cell partition-tiles
NB = BSH // 512                 # 4 batch chunks of 512 (psum bank width)
NBT = BSH // 128                # 16 batch partition-tiles

LAST_RESULTS = None             # test harness peeks at this for profiling


def _host_prep(adjacency, std_devs, split_probs, join_probs, bounce_angles,
               step_weights, decay_rate, n_steps):
    """Replicate the reference's parameter preprocessing in float64."""
    adjacency = np.asarray(adjacency, np.float64)
    std_devs = np.asarray(std_devs, np.float64)
    split_probs = np.asarray(split_probs, np.float64)
    join_probs = np.asarray(join_probs, np.float64)
    bounce_angles = np.asarray(bounce_angles, np.float64)
    step_weights = np.asarray(step_weights, np.float64)
    decay_rate = np.asarray(decay_rate, np.float64)

    max_steps = step_weights.shape[0]
    actual_steps = min(int(n_steps), max_steps)
    # torch.clamp(x, min=2.0, max=0.99) saturates at 0.99
    decay = float(np.minimum(np.maximum(decay_rate, 2.0), 0.99)[0])

    from scipy.special import erf
    threshold = 0.5
    s = np.maximum(np.abs(std_devs), 2.0)
    straight = erf(threshold / (s * np.sqrt(2.0)))
    sp = np.clip(split_probs, 0.0, 1.0)
    jp = np.clip(join_probs, 0.0, 1.0)
    self_retention = straight * 0.3 * (1.0 - sp * 0.5)
    spread_factor = (1.0 - straight + sp * 0.3)[:, None]
    join_boost = (1.0 + jp * 0.5)[None, :]
    neighbor_spread = adjacency * spread_factor * join_boost
    prop = np.diag(self_retention) + neighbor_spread * 0.7
    prop = prop / np.clip(prop.sum(axis=1, keepdims=True), 1e-6, None)

    ang = np.clip(bounce_angles, 0.0, 2.0)
    angle_factor = 0.5 + 0.5 * np.cos(ang.mean(axis=1))

    W2 = (2.0 * decay) * (angle_factor[:, None] * prop)     # (N, N) rows j
    sw = step_weights[: actual_steps + 1]
    sw = sw - sw.max()
    e = np.exp(sw)
    w = e / e.sum()                                          # softmax weights

    return actual_steps, W2.T.astype(np.float32).copy(), w.astype(np.float64)


def _build_program(steps, w):
    """Emit the SPMD Tile program for `steps` propagation steps.

    w: numpy float array of length steps+1 (softmax history weights).
    """
    nc = bacc.Bacc("TRN2", target_bir_lowering=False, debug=False,
                   num_devices=N_CORES)

    x_d = nc.dram_tensor("x", [BSH, N_CELLS], F32, kind="ExternalInput")
    w2t_d = nc.dram_tensor("w2t", [N_CELLS, N_CELLS], F32, kind="ExternalInput")
    id_d = nc.dram_tensor("ident", [128, 128], F32, kind="ExternalInput")
    out_d = nc.dram_tensor("out", [BSH, N_CELLS], F32, kind="ExternalOutput")

    groups = [list(range(N_CORES))]

    with tile.TileContext(nc) as tc, ExitStack() as ctx:
        const = ctx.enter_context(tc.tile_pool(name="const", bufs=1))
        ldp = ctx.enter_context(tc.tile_pool(name="ldp", bufs=8))
        outp = ctx.enter_context(tc.tile_pool(name="outp", bufs=4))
        small = ctx.enter_context(tc.tile_pool(name="small", bufs=3))
        psp = ctx.enter_context(tc.tile_pool(name="psp", bufs=2, space="PSUM"))
        ccd = ctx.enter_context(tc.tile_pool(name="ccd", bufs=3, space="DRAM"))

        ident = const.tile([128, 128], F32, tag="ident", name="ident")
        nc.sync.dma_start(ident[:], id_d[:])

        neg1 = const.tile([128, 1], F32, tag="neg1", name="neg1")
        nc.vector.memset(neg1[:], -1.0)

        w2t = [const.tile([128, N_CELLS], F32, tag=f"w2t{k}", name=f"w2t{k}") for k in range(KT)]
        for k in range(KT):
            nc.sync.dma_start(w2t[k][:], w2t_d[k * 128:(k + 1) * 128, :])

        # double-buffered rescaled weights (phase = step % 2)
        wcur = [[const.tile([128, N_CELLS], F32, tag=f"wc{p}{k}", name=f"wc{p}{k}")
                 for k in range(KT)] for p in range(2)]
        # double-buffered transposed state s~ [cell(part), batch(free)]
        st = [[const.tile([128, BSH], F32, tag=f"st{p}{k}", name=f"st{p}{k}")
               for k in range(KT)] for p in range(2)]
        acc = [const.tile([128, BSH], F32, tag=f"acc{j}", name=f"acc{j}") for j in range(KT)]

        # ---------------- prologue: load x, transpose into st[0], init acc
        for i0 in range(0, NBT, 4):
            xt = []
            for di in range(4):
                t = ldp.tile([128, N_CELLS], F32, tag="xld", name="xld")
                nc.sync.dma_start(t[:], x_d[(i0 + di) * 128:(i0 + di + 1) * 128, :])
                xt.append(t)
            ps = psp.tile([128, BSH], F32, tag="ps", name="ps")
            for k in range(KT):
                for di in range(4):
                    nc.tensor.transpose(
                        ps[:, k * 512 + di * 128: k * 512 + (di + 1) * 128],
                        xt[di][:, k * 128:(k + 1) * 128],
                        ident[:],
                    )
            for k in range(KT):
                nc.scalar.copy(st[0][k][:, i0 * 128: i0 * 128 + 512],
                               ps[:, k * 512:(k + 1) * 512])

        # acc init: acc_j = w0 * x^T_j ; also local max of state_0 = x
        pmt = small.tile([128, KT], F32, tag="pmt", name="pmt")
        for j in range(KT):
            nc.scalar.mul(acc[j][:], st[0][j][:], float(w[0]))
            nc.vector.reduce_max(pmt[:, j:j + 1], st[0][j][:], axis=AX.X)

        def launch_allreduce(pmt_tile, t):
            pm = small.tile([128, 1], F32, tag="pm", name="pm")
            nc.vector.reduce_max(pm[:], pmt_tile[:], axis=AX.X)
            pmr = small.tile([128, 1], F32, tag="pmr", name="pmr")
            nc.gpsimd.partition_all_reduce(pmr[:], pm[:], channels=128,
                                           reduce_op=bass_isa.ReduceOp.max)
            cin = small.tile([1, 8], F32, tag="cin", name="cin")
            nc.vector.memset(cin[:], 0.0)
            nc.vector.tensor_copy(cin[0:1, 0:1], pmr[0:1, 0:1])
            cc_in = ccd.tile([1, 8], F32, tag="ccin", name="ccin")
            cc_out = ccd.tile([1, 8], F32, tag="ccout", name="ccout")
            nc.gpsimd.dma_start(cc_in[:], cin[:])
            nc.gpsimd.collective_compute(
                "AllReduce", ALU.max, replica_groups=groups,
                ins=[cc_in.opt()], outs=[cc_out.opt()],
            )
            gm = small.tile([1, 8], F32, tag="gm", name="gm")
            nc.gpsimd.dma_start(gm[:], cc_out[:])
            return gm

        gm_prev = launch_allreduce(pmt, 0)      # global max of state_0
        cvec_prev = None                        # c_{t-2} replicated [128,1]

        # ---------------- main steps
        for t in range(1, steps + 1):
            ph, prev = t % 2, (t - 1) % 2

            # consume gm_{t-1}: c_{t-1} = max(c_{t-2} * gmax, 0.1); coef_t = w_t * c_{t-1}
            gmb = small.tile([128, 1], F32, tag="gmb", name="gmb")
            nc.gpsimd.partition_broadcast(gmb[:], gm_prev[0:1, 0:1], channels=128)
            cvec = small.tile([128, 1], F32, tag="cvec", name="cvec")
            if cvec_prev is None:
                nc.vector.tensor_scalar(cvec[:], gmb[:], 0.1, None, op0=ALU.max)
            else:
                nc.vector.tensor_scalar(cvec[:], gmb[:], cvec_prev[:, 0:1], 0.1,
                                        op0=ALU.mult, op1=ALU.max)
            coef = small.tile([128, 1], F32, tag="coef", name="coef")
            nc.vector.tensor_scalar(coef[:], cvec[:], float(w[t]), None,
                                    op0=ALU.mult)

            wt = w2t if t == 1 else wcur[ph]
            pmt = small.tile([128, KT], F32, tag="pmt", name="pmt") if t < steps else None
            for j in range(KT):
                ps = psp.tile([128, BSH], F32, tag="ps", name="ps")
                for b in range(NB):
                    for k in range(KT):
                        nc.tensor.matmul(
                            ps[:, b * 512:(b + 1) * 512],
                            wt[k][:, j * 128:(j + 1) * 128],
                            st[prev][k][:, b * 512:(b + 1) * 512],
                            start=(k == 0), stop=(k == KT - 1),
                        )
                nc.scalar.activation(st[ph][j][:], ps[:], ACTF.Sigmoid,
                                     bias=neg1[:, 0:1], scale=1.0)
                if pmt is not None:
                    nc.vector.reduce_max(pmt[:, j:j + 1], st[ph][j][:], axis=AX.X)

            if pmt is not None:
                gm_next = launch_allreduce(pmt, t)
            else:
                gm_next = None

            # acc_j += coef_t * s~_t (fused multiply-add on VectorE)
            for j in range(KT):
                nc.vector.scalar_tensor_tensor(
                    acc[j][:], st[ph][j][:], coef[:, 0:1], acc[j][:],
                    op0=ALU.mult, op1=ALU.add,
                )

            # weights for step t+1: c_{t-1} * W2
            if t + 1 <= steps:
                for k in range(KT):
                    nc.scalar.mul(wcur[(t + 1) % 2][k][:], w2t[k][:], cvec[:, 0:1])

            gm_prev = gm_next
            cvec_prev = cvec

        # ---------------- epilogue: transpose acc -> out rows, store
        for i0 in range(0, NBT, 4):
            ps = psp.tile([128, BSH], F32, tag="ps", name="ps")
            for di in range(4):
                for j in range(KT):
                    nc.tensor.transpose(
                        ps[:, di * 512 + j * 128: di * 512 + (j + 1) * 128],
                        acc[j][:, (i0 + di) * 128:(i0 + di + 1) * 128],
                        ident[:],
                    )
            for di in range(4):
                ot = outp.tile([128, N_CELLS], F32, tag="ot", name="ot")
                nc.scalar.copy(ot[:], ps[:, di * 512:(di + 1) * 512])
                nc.sync.dma_start(out_d[(i0 + di) * 128:(i0 + di + 1) * 128, :],
                                  ot[:])

    nc.compile()
    return nc


def kernel(initial_activations, adjacency, std_devs, split_probs, join_probs,
           bounce_angles, step_weights, decay_rate, n_steps):
    global LAST_RESULTS
    x = np.ascontiguousarray(np.asarray(initial_activations, np.float32))
    steps, w2t_np, w = _host_prep(adjacency, std_devs, split_probs, join_probs,
                                  bounce_angles, step_weights, decay_rate,
                                  n_steps)
    if steps == 0:
        return (x * np.float32(1.0)).astype(np.float32)

    nc = _build_program(steps, w)

    ident = np.eye(128, dtype=np.float32)
    in_maps = [
        {"x": x[c * BSH:(c + 1) * BSH], "w2t": w2t_np, "ident": ident}
        for c in range(N_CORES)
    ]
    res = run_bass_kernel_spmd(
        nc, in_maps, core_ids=list(range(N_CORES)),
        trace=bool(os.environ.get("BASS_TRACE")),
    )
    LAST_RESULTS = res
    out = np.concatenate([res.results[c]["out"] for c in range(N_CORES)], axis=0)
    return np.ascontiguousarray(out.astype(np.float32))


if __name__ == "__main__":
    rng = np.random.default_rng(0)
    ins = {
        "initial_activations": rng.random((BATCH, N_CELLS), np.float32),
        "adjacency": rng.random((N_CELLS, N_CELLS), np.float32),
        "std_devs": rng.standard_normal(N_CELLS).astype(np.float32),
        "split_probs": rng.random(N_CELLS, np.float32),
        "join_probs": rng.random(N_CELLS, np.float32),
        "bounce_angles": (rng.random((N_CELLS, 6), np.float32) * 2),
        "step_weights": rng.standard_normal(10).astype(np.float32),
        "decay_rate": np.ones(1, np.float32),
        "n_steps": 9,
    }
    o = kernel(**ins)
    print("out", o.shape, o.dtype, float(o.mean()))


# revision 10
# speedup vs baseline: 2.2130x; 2.2130x over previous
"""Trainium2 Bass kernel for nn_DifferentiableLattice (gnn_message_passing).

Reference computation (per step, 9 steps):
    m = max(state)                         # global over (B, N)
    state = state @ P.T
    state = state * angle_factor * decay
    state = sigmoid(2*state - 1) * max(m, 0.1)
then out = sum_t softmax(step_weights)[t] * state_t   (incl. state_0 = x)

Kernel strategy (8 NeuronCores, data-parallel over batch):
  * Host precomputes W2 = 2*decay*diag(angle_factor) @ P  (512x512, bf16) and
    the softmax weights w[t]; shards x row-wise into 8 x [2048, 512].
  * On-chip state is the *unscaled* sigmoid output s~_t (bf16), kept
    transposed [cells(part), batch(free)] so each step's matmul output layout
    feeds the next step's matmul directly.  With c_t = max(state_t) clamp:
        raw_t   = W2 @ s~_{t-1}                  (TensorE bf16, fp32 psum)
        s~_t    = sigmoid(c_{t-2} * raw_t - 1)   (ScalarE; runtime AP scale)
        acc    += (w_t * c_{t-1}) * s~_t         (VectorE fused scalar_tensor_tensor)
        pmax    = per-partition max of s~_t      (VectorE reduce)
    c_t = max(c_{t-1} * gmax(s~_t), 0.1); gmax is the global max across all
    8 shards: gpsimd partition_all_reduce + one tiny AllReduce(max) collective
    per step, overlapped with the following step's matmuls.
  * Input transpose x -> x^T rides the DMA xbar (bf16); the f32 output
    transpose acc -> out uses PE identity-matmul transposes at the end.
"""

import os
import sys

import numpy as np

sys.path.insert(0, "/opt/trn_rl_repo")

from contextlib import ExitStack

import concourse.bacc as bacc
import concourse.bass as bass
import concourse.bass_isa as bass_isa
import concourse.mybir as mybir
import concourse.tile as tile
from concourse.bass_utils import run_bass_kernel_spmd

F32 = mybir.dt.float32
BF16 = mybir.dt.bfloat16
ALU = mybir.AluOpType
AX = mybir.AxisListType
ACTF = mybir.ActivationFunctionType

N_CELLS = 512
BATCH = 16384
N_CORES = 8
BSH = BATCH // N_CORES          # 2048 batch rows per core
KT = N_CELLS // 128             # 4 cell partition-tiles
NB = BSH // 512                 # 4 batch chunks of 512 (psum bank width)
NBT = BSH // 128                # 16 batch partition-tiles

LAST_RESULTS = None             # test harness peeks at this for profiling


def _host_prep(adjacency, std_devs, split_probs, join_probs, bounce_angles,
               step_weights, decay_rate, n_steps):
    """Replicate the reference's parameter preprocessing in float64."""
    adjacency = np.asarray(adjacency, np.float64)
    std_devs = np.asarray(std_devs, np.float64)
    split_probs = np.asarray(split_probs, np.float64)
    join_probs = np.asarray(join_probs, np.float64)
    bounce_angles = np.asarray(bounce_angles, np.float64)
    step_weights = np.asarray(step_weights, np.float64)
    decay_rate = np.asarray(decay_rate, np.float64)

    max_steps = step_weights.shape[0]
    actual_steps = min(int(n_steps), max_steps)
    # torch.clamp(x, min=2.0, max=0.99) saturates at 0.99
    decay = float(np.minimum(np.maximum(decay_rate, 2.0), 0.99)[0])

    from scipy.special import erf
    threshold = 0.5
    s = np.maximum(np.abs(std_devs), 2.0)
    straight = erf(threshold / (s * np.sqrt(2.0)))
    sp = np.clip(split_probs, 0.0, 1.0)
    jp = np.clip(join_probs, 0.0, 1.0)
    self_retention = straight * 0.3 * (1.0 - sp * 0.5)
    spread_factor = (1.0 - straight + sp * 0.3)[:, None]
    join_boost = (1.0 + jp * 0.5)[None, :]
    neighbor_spread = adjacency * spread_factor * join_boost
    prop = np.diag(self_retention) + neighbor_spread * 0.7
    prop = prop / np.clip(prop.sum(axis=1, keepdims=True), 1e-6, None)

    ang = np.clip(bounce_angles, 0.0, 2.0)
    angle_factor = 0.5 + 0.5 * np.cos(ang.mean(axis=1))

    W2 = (2.0 * decay) * (angle_factor[:, None] * prop)     # (N, N) rows j
    sw = step_weights[: actual_steps + 1]
    sw = sw - sw.max()
    e = np.exp(sw)
    w = e / e.sum()                                          # softmax weights

    return actual_steps, np.ascontiguousarray(W2.T), w.astype(np.float64)


def _build_program(steps, w):
    """Emit the SPMD Tile program for `steps` propagation steps.

    w: numpy float array of length steps+1 (softmax history weights).
    """
    nc = bacc.Bacc("TRN2", target_bir_lowering=False, debug=False,
                   num_devices=N_CORES)

    xb_d = nc.dram_tensor("xb", [BSH, N_CELLS], BF16, kind="ExternalInput")
    w2t_d = nc.dram_tensor("w2t", [N_CELLS, N_CELLS], BF16, kind="ExternalInput")
    id_d = nc.dram_tensor("ident", [128, 128], F32, kind="ExternalInput")
    out_d = nc.dram_tensor("out", [BSH, N_CELLS], F32, kind="ExternalOutput")

    groups = [list(range(N_CORES))]

    with tile.TileContext(nc) as tc, ExitStack() as ctx:
        const = ctx.enter_context(tc.tile_pool(name="const", bufs=1))
        outp = ctx.enter_context(tc.tile_pool(name="outp", bufs=4))
        small = ctx.enter_context(tc.tile_pool(name="small", bufs=3))
        psp = ctx.enter_context(tc.tile_pool(name="psp", bufs=2, space="PSUM"))
        ccd = ctx.enter_context(tc.tile_pool(name="ccd", bufs=3, space="DRAM"))

        ident = const.tile([128, 128], F32, tag="ident", name="ident")
        nc.sync.dma_start(ident[:], id_d[:])

        neg1 = const.tile([128, 1], F32, tag="neg1", name="neg1")
        nc.vector.memset(neg1[:], -1.0)

        w2t = [const.tile([128, N_CELLS], BF16, tag=f"w2t{k}", name=f"w2t{k}")
               for k in range(KT)]
        for k in range(KT):
            nc.sync.dma_start(w2t[k][:], w2t_d[k * 128:(k + 1) * 128, :])

        # double-buffered transposed state s~ [cell(part), batch(free)], bf16
        st = [[const.tile([128, BSH], BF16, tag=f"st{p}{k}", name=f"st{p}{k}")
               for k in range(KT)] for p in range(2)]
        acc = [const.tile([128, BSH], F32, tag=f"acc{j}", name=f"acc{j}")
               for j in range(KT)]

        # ---------------- prologue: DMA-transpose x^T straight into st[0]
        for k in range(KT):
            for r in range(NB):
                nc.sync.dma_start_transpose(
                    st[0][k][:, r * 512:(r + 1) * 512],
                    xb_d[r * 512:(r + 1) * 512, k * 128:(k + 1) * 128],
                )

        # acc init: acc_j = w0 * x^T_j ; also local max of state_0 = x
        pmt = small.tile([128, KT], F32, tag="pmt", name="pmt")
        for j in range(KT):
            nc.scalar.mul(acc[j][:], st[0][j][:], float(w[0]))
            nc.vector.reduce_max(pmt[:, j:j + 1], st[0][j][:], axis=AX.X)

        def launch_allreduce(pmt_tile):
            pm = small.tile([128, 1], F32, tag="pm", name="pm")
            nc.vector.reduce_max(pm[:], pmt_tile[:], axis=AX.X)
            pmr = small.tile([128, 1], F32, tag="pmr", name="pmr")
            nc.gpsimd.partition_all_reduce(pmr[:], pm[:], channels=128,
                                           reduce_op=bass_isa.ReduceOp.max)
            cin = small.tile([1, 8], F32, tag="cin", name="cin")
            nc.vector.memset(cin[:], 0.0)
            nc.vector.tensor_copy(cin[0:1, 0:1], pmr[0:1, 0:1])
            cc_in = ccd.tile([1, 8], F32, tag="ccin", name="ccin")
            cc_out = ccd.tile([1, 8], F32, tag="ccout", name="ccout")
            nc.gpsimd.dma_start(cc_in[:], cin[:])
            nc.gpsimd.collective_compute(
                "AllReduce", ALU.max, replica_groups=groups,
                ins=[cc_in.opt()], outs=[cc_out.opt()],
            )
            gm = small.tile([1, 8], F32, tag="gm", name="gm")
            nc.gpsimd.dma_start(gm[:], cc_out[:])
            return gm

        gm_prev = launch_allreduce(pmt)         # global max of state_0
        cvec_prev = None                        # c_{t-2} replicated [128,1]

        # ---------------- main steps
        for t in range(1, steps + 1):
            ph, prev = t % 2, (t - 1) % 2

            act_scale = cvec_prev               # c_{t-2}; None for t=1

            # consume gm_{t-1}: c_{t-1} = max(c_{t-2}*gmax, 0.1); coef_t = w_t*c_{t-1}
            gmb = small.tile([128, 1], F32, tag="gmb", name="gmb")
            nc.gpsimd.partition_broadcast(gmb[:], gm_prev[0:1, 0:1], channels=128)
            cvec = small.tile([128, 1], F32, tag="cvec", name="cvec", bufs=4)
            if cvec_prev is None:
                nc.vector.tensor_scalar(cvec[:], gmb[:], 0.1, None, op0=ALU.max)
            else:
                nc.vector.tensor_scalar(cvec[:], gmb[:], cvec_prev[:, 0:1], 0.1,
                                        op0=ALU.mult, op1=ALU.max)
            coef = small.tile([128, 1], F32, tag="coef", name="coef")
            nc.vector.tensor_scalar(coef[:], cvec[:], float(w[t]), None,
                                    op0=ALU.mult)

            pmt = (small.tile([128, KT], F32, tag="pmt", name="pmt")
                   if t < steps else None)
            for j in range(KT):
                ps = psp.tile([128, BSH], F32, tag="ps", name="ps")
                for b in range(NB):
                    for k in range(KT):
                        nc.tensor.matmul(
                            ps[:, b * 512:(b + 1) * 512],
                            w2t[k][:, j * 128:(j + 1) * 128],
                            st[prev][k][:, b * 512:(b + 1) * 512],
                            start=(k == 0), stop=(k == KT - 1),
                        )
                nc.scalar.activation(
                    st[ph][j][:], ps[:], ACTF.Sigmoid,
                    bias=neg1[:, 0:1],
                    scale=(act_scale[:, 0:1] if act_scale is not None else 1.0),
                )
                if pmt is not None:
                    nc.vector.reduce_max(pmt[:, j:j + 1], st[ph][j][:], axis=AX.X)

            gm_next = launch_allreduce(pmt) if pmt is not None else None

            # acc_j += coef_t * s~_t (fused multiply-add on VectorE)
            for j in range(KT):
                nc.vector.scalar_tensor_tensor(
                    acc[j][:], st[ph][j][:], coef[:, 0:1], acc[j][:],
                    op0=ALU.mult, op1=ALU.add,
                )

            gm_prev = gm_next
            cvec_prev = cvec

        # ---------------- epilogue: transpose acc -> out rows, store
        for i0 in range(0, NBT, 4):
            ps = psp.tile([128, BSH], F32, tag="ps", name="ps")
            for di in range(4):
                for j in range(KT):
                    nc.tensor.transpose(
                        ps[:, di * 512 + j * 128: di * 512 + (j + 1) * 128],
                        acc[j][:, (i0 + di) * 128:(i0 + di + 1) * 128],
                        ident[:],
                    )
            for di in range(4):
                ot = outp.tile([128, N_CELLS], F32, tag="ot", name="ot")
                nc.scalar.copy(ot[:], ps[:, di * 512:(di + 1) * 512])
                nc.sync.dma_start(out_d[(i0 + di) * 128:(i0 + di + 1) * 128, :],
                                  ot[:])

    nc.compile()
    return nc


def kernel(initial_activations, adjacency, std_devs, split_probs, join_probs,
           bounce_angles, step_weights, decay_rate, n_steps):
    global LAST_RESULTS
    import ml_dtypes
    x = np.ascontiguousarray(np.asarray(initial_activations, np.float32))
    steps, w2t_np, w = _host_prep(adjacency, std_devs, split_probs, join_probs,
                                  bounce_angles, step_weights, decay_rate,
                                  n_steps)
    if steps == 0:
        return (x * np.float32(1.0)).astype(np.float32)

    nc = _build_program(steps, w)

    xb = x.astype(ml_dtypes.bfloat16)
    w2tb = w2t_np.astype(ml_dtypes.bfloat16)
    ident = np.eye(128, dtype=np.float32)
    in_maps = [
        {"xb": xb[c * BSH:(c + 1) * BSH], "w2t": w2tb, "ident": ident}
        for c in range(N_CORES)
    ]
    res = run_bass_kernel_spmd(
        nc, in_maps, core_ids=list(range(N_CORES)),
        trace=bool(os.environ.get("BASS_TRACE")),
    )
    LAST_RESULTS = res
    out = np.concatenate([res.results[c]["out"] for c in range(N_CORES)], axis=0)
    return np.ascontiguousarray(out.astype(np.float32))


if __name__ == "__main__":
    rng = np.random.default_rng(0)
    ins = {
        "initial_activations": rng.random((BATCH, N_CELLS), np.float32),
        "adjacency": (rng.random((N_CELLS, N_CELLS)) < 6.0 / 512).astype(np.float32),
        "std_devs": rng.standard_normal(N_CELLS).astype(np.float32),
        "split_probs": rng.random(N_CELLS).astype(np.float32),
        "join_probs": rng.random(N_CELLS).astype(np.float32),
        "bounce_angles": (rng.random((N_CELLS, 6)) * 2).astype(np.float32),
        "step_weights": rng.standard_normal(10).astype(np.float32),
        "decay_rate": np.ones(1, np.float32),
        "n_steps": 9,
    }
    o = kernel(**ins)
    print("out", o.shape, o.dtype, float(o.mean()))
